# revision 50
# baseline (speedup 1.0000x reference)
"""Trainium2 Bass kernel for nn_AutoregressivePrior.

Computes a K-step tiny-LSTM autoregressive prior (HID=256), projects each
step's hidden state to (loc, scale) rows of width 64, and materializes the
batch-broadcast output [K*batch_size, 64] for both loc and scale.

Strategy (8 NeuronCores, SPMD):
  - The LSTM recurrence + projections are tiny and replicated on every core.
  - The broadcast/repeat over batch_size (the memory-bound part) is sharded:
    each core writes its own batch_size/8 = 4096-row slice of every output
    row k, as two contiguous 0.5 MB fp16 DMAs per k.

Design notes:
  - The LSTM state lives in column layout [128 partitions, pairs]: gate
    pre-activations are computed as gates^T with the weight chunk as the
    stationary matmul operand, so every elementwise/activation op runs on
    128 lanes, and the hidden state needs no transpose between steps.
  - Every value is kept as duplicated column pairs [v0 v0 v1 v1]; h then
    comes out of the elementwise chain pre-duplicated as the next step's
    moving operand.
  - All matmul operands are fp16 (PSUM accumulation stays fp32). Non-fp32
    128-column stationaries enable the PE's fast-weight-load path, which
    is what makes the 16 LDWEIGHTS+MATMUL pairs per step cheap; fp32r
    disables FWL and measures ~4x slower per pair.
  - Gate tiles are split (g | i | f,o) so the serial chain's inputs come
    off the ACT engine as early as possible: tanh(g) and sigmoid(i) fire
    while the f/o matmuls still stream, so i*tanh(g) runs off the critical
    path; f and o share one PSUM tile, one DVE bias add, and one sigmoid.
    (Bias via contraction-1 matmuls was tried and is a net loss: 3-matmul
    PSUM accumulation groups break the PE's pipelined LDWEIGHTS cadence,
    and two simultaneously-open groups on one PSUM tile corrupt results.)
  - Outputs are written as fp16 (host widens to fp32); this halves the
    HBM write traffic, which is the roofline for this problem. End-to-end
    error vs the fp32 reference is ~1e-3, comfortably inside the 2e-2 gate.
  - After step 0, x and h are both h_new, so gates = (W_ih + W_hh) @ h + b.
  - Queue split: the sync-engine HWDGE ring carries only the tiny megaA
    load and then the output stores (row 0's store can flow from ~8 us);
    the scalar ring carries the gate weights, each split into gate-major
    halves so step 1's leading chunks land ~0.8 us earlier. The
    activation-table warm-up (a sigmoid on a zeroed tile; its table set
    also contains tanh) is emitted after the weight triggers so the
    ~1.3 us table DMAs occupy the scalar engine only once triggers are out.
  - Each output row's store repeats a small [128, 8x64] fp16 SBUF block
    per tensor via the DMA read-AP (1 KB descriptors; descriptor metadata
    costs ~3% of HBM-side bandwidth, 512 B descriptors would cost ~2x
    that). The two per-tensor widen DVE ops are emitted after the current
    step's gate chain, filling the DVE idle window during the next step's
    matmul stream instead of stalling the serial chain.
"""

import numpy as np

import concourse.bacc as bacc
import concourse.mybir as mybir
from concourse.tile import TileContext
from concourse.bass_utils import run_bass_kernel_spmd

F32 = mybir.dt.float32
F16 = mybir.dt.float16

HID = 256
K = 7
BATCH = 32768
NCORES = 8
BS = BATCH // NCORES  # 4096 batch rows per core
P = 128               # partitions
RPP = BS // P         # 32 batch rows per partition
ZM = 64               # zm_size
REP_SB = 8            # batch-row copies materialized in SBUF per output row
REP_DMA = RPP // REP_SB  # additional repeats done by the store's read-AP

# --- packed const layouts (column offsets) ---
# megaA (f16): projection weights + input column
MA_WL = 0                  # wlst chunks (c p n): cols [0, 256)
MA_ZC = 256                # zm1 column form, duplicated pairs: [256, 260)
MA_W = 260
# megaF (f32): gate bias columns (g | i f | o) + projection bias broadcast
MF_BG = 0                  # g bias, duplicated pairs: [0, 4)
MF_BIF = 4                 # i|f bias, duplicated pairs: [4, 12)
MF_BO = 12                 # o bias, duplicated pairs: [12, 16)
MF_BLSB = 16               # biasls broadcast to all partitions: [16, 144)
MF_W = 144
# megaB1/megaB2 (f16): step-1 / steady-state weights, gate-major chunks:
# chunk (c, m) at cols m*256 + c*128, so cols [0,1024) serve gates g,i and
# cols [1024,2048) serve f,o (each mega is loaded as two DMA halves)
MB_W = 2048

_NC_CACHE = {}


def build_nc():
    nc = bacc.Bacc("TRN2", target_bir_lowering=False, debug=False)

    megaA_d = nc.declare_dram_parameter("megaA", [P, MA_W], F16, isOutput=False)
    megaF_d = nc.declare_dram_parameter("megaF", [P, MF_W], F32, isOutput=False)
    megaB1_d = nc.declare_dram_parameter("megaB1", [P, MB_W], F16, isOutput=False)
    megaB2_d = nc.declare_dram_parameter("megaB2", [P, MB_W], F16, isOutput=False)
    out_d = nc.declare_dram_parameter("out", [K, 2, BS, ZM], F16, isOutput=True)

    with TileContext(nc) as tc:
        with (
            tc.tile_pool(name="const", bufs=1) as cpool,
            tc.tile_pool(name="state", bufs=3) as spool,
            tc.tile_pool(name="hcol", bufs=3) as hpool,
            tc.tile_pool(name="wide", bufs=8) as wpool,
            tc.tile_pool(name="rowv", bufs=3) as rpool,
            tc.tile_pool(name="pgg", bufs=1, space="PSUM") as pgg_pool,
            tc.tile_pool(name="pgif", bufs=1, space="PSUM") as pgif_pool,
            tc.tile_pool(name="pgo", bufs=1, space="PSUM") as pgo_pool,
            tc.tile_pool(name="pbcast", bufs=2, space="PSUM") as pb_pool,
        ):
            # sync ring: only the small projection/bias const tile, so the
            # output stores behind it in the queue start flowing early
            ma = cpool.tile([P, MA_W], F16)
            nc.sync.dma_start(out=ma[:], in_=megaA_d[:])
            # scalar ring: projection bias, then the two gate-weight megas,
            # each as two gate-major halves (g,i first)
            mf = cpool.tile([P, MF_W], F32)
            nc.scalar.dma_start(out=mf[:], in_=megaF_d[:])
            mb1 = cpool.tile([P, MB_W], F16)
            nc.scalar.dma_start(out=mb1[:, 0:1024], in_=megaB1_d[:, 0:1024])
            nc.scalar.dma_start(out=mb1[:, 1024:2048], in_=megaB1_d[:, 1024:2048])
            mb2 = cpool.tile([P, MB_W], F16)
            nc.scalar.dma_start(out=mb2[:, 0:1024], in_=megaB2_d[:, 0:1024])
            nc.scalar.dma_start(out=mb2[:, 1024:2048], in_=megaB2_d[:, 1024:2048])

            wlst_sb = ma[:, MA_WL : MA_WL + 256]
            zm1c_a = ma[:, MA_ZC : MA_ZC + 4]
            bg_g = mf[:, MF_BG : MF_BG + 4]
            bg_if = mf[:, MF_BIF : MF_BIF + 8]
            bg_o = mf[:, MF_BO : MF_BO + 4]
            blsb = mf[:, MF_BLSB : MF_BLSB + 128]

            # Warm the activation-table SRAM while the weights stream in:
            # sigmoid lives only in the `sigmoid_and_others` set, which also
            # contains tanh, so one front-loaded table serves every later
            # activation instead of a lazy ~1.3us load landing mid-step-1.
            # (The zero-fill must be a DVE memset: a Copy-activation memzero
            # would itself pull in a tanh-less table set first.)
            warm = spool.tile([P, 4], F32)
            nc.vector.memset(warm[:], 0.0)
            nc.scalar.activation(
                out=warm[:], in_=warm[:],
                func=mybir.ActivationFunctionType.Sigmoid,
            )

            def proj_mm(xcr):
                """Project p_z[k] (f16 column form xcr) to loc|scale in PSUM.

                The projection and 128-partition broadcast are fused into one
                PSUM accumulation group: the x column is replicated across the
                stationary operand's free dim, so every output partition
                computes the same (loc | scale) row. Row k's projection is
                emitted after step k+1's gate matmuls (and deliberately NOT
                at high priority), so the serial gate stream -- which the
                whole schedule paces on -- is never delayed behind it."""
                pb = pb_pool.tile([P, 2 * ZM], F32)
                with tc.high_priority():
                    nc.tensor.matmul(
                        pb[:], lhsT=xcr[:, 0:1].broadcast_to((P, P)),
                        rhs=wlst_sb[:, 0:128], start=True, stop=False,
                    )
                    nc.tensor.matmul(
                        pb[:], lhsT=xcr[:, 2:3].broadcast_to((P, P)),
                        rhs=wlst_sb[:, 128:256], start=False, stop=True,
                    )
                return pb

            def flush_row(k, pb):
                """Widen row k's projection PSUM into the fp16 DMA staging
                block (adding the projection bias) and store this core's
                batch-broadcast slice.

                Two per-tensor DVE ops instead of one long one: the caller
                orders them after the current step's gate-chain TTs, so they
                fill the DVE idle window during the next step's matmul
                stream instead of stalling the serial chain. The store's
                read-AP repeats the block REP_DMA more times as 1 KB
                descriptors."""
                rowv = rpool.tile([P, 2 * ZM], F16)
                nc.vector.tensor_add(out=rowv[:], in0=pb[:], in1=blsb)
                for t in (0, 1):
                    midw = wpool.tile([P, REP_SB * ZM], F16)
                    nc.vector.tensor_copy(
                        out=midw[:].rearrange("p (r j) -> p r j", r=REP_SB),
                        in_=rowv[:, t * ZM : (t + 1) * ZM][
                            :, None, :
                        ].broadcast_to((P, REP_SB, ZM)),
                    )
                    nc.sync.dma_start(
                        out=out_d[k, t].rearrange("(p r) j -> p (r j)", p=P),
                        in_=midw[:, None, :].broadcast_to(
                            (P, REP_DMA, REP_SB * ZM)
                        ),
                    )

            def gate_mm(dst, wsb, m, xcr):
                """Compute gate chunk m of W.T @ x into dst [P, 2]."""
                for c in (0, 1):
                    nc.tensor.matmul(
                        dst,
                        lhsT=wsb[:, m * 256 + c * 128 : m * 256 + (c + 1) * 128],
                        rhs=xcr[:, 2 * c : 2 * c + 2],
                        start=(c == 0), stop=(c == 1),
                    )

            def emit_step(t, xcr_prev, st_prev):
                """One LSTM cell step, duplicated-pair column layout.

                Gate chunk order (g, i, f, o). tanh(g) fires after g's 4
                matmuls; i and f share one PSUM tile / bias add / sigmoid
                that fires after f's matmuls (NOT gated on o, unlike an
                i|f|o merge), so the serial c-chain t1 -> t1+t2 -> tanh(c)
                starts while o's matmuls and sigmoid are still in flight.
                o's sigmoid -- needed only for the final h multiply -- is
                emitted before tanh(c) so the ACT FIFO runs it in the gap
                while the c-sum is still on the DVE.

                st tiles hold [tanh(g) (0:4) | c (4:8)].
                Returns (st_next, h16); h16 is [128, 4] = [h0 h0 h1 h1]."""
                wsb = mb1 if t == 1 else mb2
                pgG = pgg_pool.tile([P, 4], F32)
                pgIF = pgif_pool.tile([P, 8], F32)
                pgO = pgo_pool.tile([P, 4], F32)
                for dm in (0, 1):
                    gate_mm(pgG[:, 2 * dm : 2 * dm + 2], wsb, dm, xcr_prev)
                for dm in range(4):
                    gate_mm(pgIF[:, 2 * dm : 2 * dm + 2], wsb, 2 + dm, xcr_prev)
                for dm in (0, 1):
                    gate_mm(pgO[:, 2 * dm : 2 * dm + 2], wsb, 6 + dm, xcr_prev)

                bg = spool.tile([P, 4], F32)
                nc.vector.tensor_add(out=bg[:], in0=pgG[:], in1=bg_g)
                nc.scalar.activation(
                    out=st_prev[:, 0:4], in_=bg[:],
                    func=mybir.ActivationFunctionType.Tanh,
                )
                bif = spool.tile([P, 8], F32)
                nc.vector.tensor_add(out=bif[:], in0=pgIF[:], in1=bg_if)
                sif = spool.tile([P, 8], F32)
                nc.scalar.activation(
                    out=sif[:], in_=bif[:],
                    func=mybir.ActivationFunctionType.Sigmoid,
                )
                si, sf = sif[:, 0:4], sif[:, 4:8]
                bo = spool.tile([P, 4], F32)
                nc.vector.tensor_add(out=bo[:], in0=pgO[:], in1=bg_o)
                so = spool.tile([P, 4], F32)
                nc.scalar.activation(
                    out=so[:], in_=bo[:],
                    func=mybir.ActivationFunctionType.Sigmoid,
                )
                st_next = spool.tile([P, 8], F32, tag="st")
                if t == 1:
                    # c0 = 0: c1 = i*tanh(g) directly into st_next's c half
                    nc.vector.tensor_mul(
                        out=st_next[:, 4:8], in0=si, in1=st_prev[:, 0:4]
                    )
                else:
                    t1 = spool.tile([P, 4], F32)
                    nc.vector.tensor_mul(out=t1[:], in0=si, in1=st_prev[:, 0:4])
                    t2 = spool.tile([P, 4], F32)
                    nc.vector.tensor_mul(out=t2[:], in0=sf, in1=st_prev[:, 4:8])
                    nc.vector.tensor_add(out=st_next[:, 4:8], in0=t1[:], in1=t2[:])
                tc_ = spool.tile([P, 4], F32)
                nc.scalar.activation(
                    out=tc_[:], in_=st_next[:, 4:8],
                    func=mybir.ActivationFunctionType.Tanh,
                )
                h16 = hpool.tile([P, 4], F16)
                nc.vector.tensor_mul(out=h16[:], in0=so, in1=tc_[:])
                return st_next, h16

            # Software pipeline, lagged one step: row k's projection matmuls
            # are emitted after step k+1's gate matmuls (PE FIFO: the serial
            # gate stream always leads), and row k's flush (DVE widen +
            # store) one step later still, so its DVE ops run in the idle
            # window during a later matmul stream. Row 0 is fully eager (PE
            # and DVE are idle while step 1's weights are still in flight).
            pb_prev = proj_mm(zm1c_a)
            flush_row(0, pb_prev)
            pb_prev = None
            xcr = zm1c_a
            st = spool.tile([P, 8], F32, tag="st")
            for t in range(1, K):
                st, xcr = emit_step(t, xcr, st)
                if pb_prev is not None:
                    flush_row(t - 1, pb_prev)
                pb_prev = proj_mm(xcr)
            flush_row(K - 1, pb_prev)

    nc.compile()
    return nc


def _get_nc():
    if "nc" not in _NC_CACHE:
        _NC_CACHE["nc"] = build_nc()
    return _NC_CACHE["nc"]


def prepare_inputs(**inputs):
    """Host-side prep: pure numpy reshuffling of the full inputs into the
    per-core input map (identical on every core)."""
    f = lambda k: np.asarray(inputs[k], dtype=np.float32)
    zm_1, W_ih, W_hh = f("zm_1"), f("W_ih"), f("W_hh")
    b_ih, b_hh = f("b_ih"), f("b_hh")
    W_loc, b_loc, W_scale, b_scale = f("W_loc"), f("b_loc"), f("W_scale"), f("b_scale")
    assert int(inputs["K"]) == K and int(inputs["batch_size"]) == BATCH

    def cpn(wt):
        # [256, N] -> chunked [128, 2*N]: chunk c (rows c*128..) at cols [c*N, (c+1)*N)
        n = wt.shape[1]
        return wt.reshape(2, P, n).transpose(1, 0, 2).reshape(P, 2 * n)

    def cpn_gate_major(wt):
        # [256, 1024] -> [128, 2048] with chunk (c, m) at cols m*256 + c*128
        return wt.reshape(2, P, 8, P).transpose(1, 2, 0, 3).reshape(P, 2048)

    # reorder gates (i, f, g, o) -> (g, i, f, o): g starts the serial chain,
    # i and f pipeline behind it, o overlaps the c chain
    perm = np.r_[512:768, 0:256, 256:512, 768:1024]
    w0t = W_ih[perm].T                 # [256, 1024]
    wst = (W_ih + W_hh)[perm].T        # [256, 1024]
    biasg = (b_ih + b_hh)[perm]        # [1024]
    wlst = np.concatenate([W_loc.T, W_scale.T], axis=1)  # [256, 128]
    biasls = np.concatenate([b_loc, b_scale])            # [128]
    zm1c = zm_1.reshape(2, P).T                          # [128, 2]
    zm1c_dup = np.repeat(zm1c, 2, axis=1)                # [128, 4]

    ma = np.zeros((P, MA_W), np.float16)
    ma[:, MA_WL : MA_WL + 256] = cpn(wlst).astype(np.float16)
    ma[:, MA_ZC : MA_ZC + 4] = zm1c_dup.astype(np.float16)

    mf = np.zeros((P, MF_W), np.float32)
    # gate bias as duplicated-pair columns, (g | i f o) matching the tiles
    mf[:, MF_BG : MF_BG + 16] = np.repeat(biasg.reshape(8, P).T, 2, axis=1)
    mf[:, MF_BLSB : MF_BLSB + 128] = biasls[None, :]

    mb1 = cpn_gate_major(w0t).astype(np.float16)
    mb2 = cpn_gate_major(wst).astype(np.float16)

    return {"megaA": ma, "megaF": mf, "megaB1": mb1, "megaB2": mb2}


def execute(in_map, **kwargs):
    nc = _get_nc()
    return run_bass_kernel_spmd(
        nc, [dict(in_map) for _ in range(NCORES)], core_ids=list(range(NCORES)), **kwargs
    )


def assemble_output(results):
    loc = np.empty((K, BATCH, ZM), np.float32)
    scale = np.empty((K, BATCH, ZM), np.float32)
    for c in range(NCORES):
        o = results[c]["out"]  # [K, 2, BS, ZM] fp16
        loc[:, c * BS : (c + 1) * BS] = o[:, 0]
        scale[:, c * BS : (c + 1) * BS] = o[:, 1]
    return loc.reshape(-1, ZM), scale.reshape(-1, ZM)


def kernel(**inputs):
    in_map = prepare_inputs(**inputs)
    res = execute(in_map)
    return assemble_output(res.results)
